# revision 1
# baseline (speedup 1.0000x reference)
"""Grouped-Query Attention kernel for Trainium2, 8-core SPMD.

Problem (full shapes): B=2, S=2048, D=2048, H=32 q-heads, KV=8 kv-heads,
DK=64, REP=4.

Sharding: 16 (batch, kv-group) units over 8 cores -> each core owns one
batch b and 2 adjacent kv-groups (8 query heads, 512 q-cols / 128 kv-cols).
Each core computes its heads' attention output and a partial output
projection against its 512-row slice of Wo; the host sums the 4 partials
per batch and adds bo.

Everything on-chip lives in "transposed" space (feature dim on SBUF
partitions): the host passes x pre-transposed (xT = x[b].T) so no on-chip
transposition of activations is needed, scores are computed directly as
P^T = exp((K^T)^T-style matmuls with t on PSUM partitions, and the final
output is produced as outT = Wo_slice^T @ attn_out^T, un-transposed on the
host.
"""

import os
from contextlib import ExitStack

import numpy as np

import concourse.bass as bass
import concourse.tile as tile
from concourse import bacc
from concourse import mybir
from concourse.masks import make_identity

F32 = mybir.dt.float32
F16 = mybir.dt.float16

# Full-problem constants (hardcoded per contest contract).
B = 2
S = 2048
D = 2048
H = 32
KV = 8
DK = 64
REP = H // KV          # 4
NCORES = 8

GPC = (KV * B) // NCORES      # kv-groups per core = 2
QC = GPC * REP * DK           # local q cols = 512
KC = GPC * DK                 # local k cols = 128
HL = GPC * REP                # local heads = 8
SB = 512                      # s-block size
NB = S // SB                  # 4 blocks
NKD = D // 128                # 16 contraction chunks for projections
NQT = QC // 128               # 4 q-col tiles
NPR = QC // 128               # 4 head-pair tiles (rhs chunks for out proj)
NOT = D // 128                # 16 out-col tiles
TPB = SB // 128               # 4 t-chunks per s-block

NEG = -1.0e30


def build_gqa_nc():
    nc = bacc.Bacc("TRN2", target_bir_lowering=False, debug=False)

    xT = nc.dram_tensor("xT", [D, S], F16, kind="ExternalInput").ap()
    wq = nc.dram_tensor("wq", [D, QC], F16, kind="ExternalInput").ap()
    wk = nc.dram_tensor("wk", [D, KC], F16, kind="ExternalInput").ap()
    wv = nc.dram_tensor("wv", [D, KC], F16, kind="ExternalInput").ap()
    wo = nc.dram_tensor("wo", [QC, D], F16, kind="ExternalInput").ap()
    bq = nc.dram_tensor("bq", [QC], F32, kind="ExternalInput").ap()
    bk = nc.dram_tensor("bk", [KC], F32, kind="ExternalInput").ap()
    bv = nc.dram_tensor("bv", [KC], F32, kind="ExternalInput").ap()
    outT = nc.dram_tensor("outT", [D, S], F32, kind="ExternalOutput").ap()

    with tile.TileContext(nc) as tc, ExitStack() as ctx:
        singles = ctx.enter_context(tc.tile_pool(name="singles", bufs=1))
        wpool = ctx.enter_context(tc.tile_pool(name="wpool", bufs=1))
        xtp = ctx.enter_context(tc.tile_pool(name="xtp", bufs=2))
        qtp = ctx.enter_context(tc.tile_pool(name="qtp", bufs=2))
        vtp = ctx.enter_context(tc.tile_pool(name="vtp", bufs=2))
        ptp = ctx.enter_context(tc.tile_pool(name="ptp", bufs=8))
        atp = ctx.enter_context(tc.tile_pool(name="atp", bufs=2))
        otp = ctx.enter_context(tc.tile_pool(name="otp", bufs=3))
        smp = ctx.enter_context(tc.tile_pool(name="smp", bufs=4))

        pp_pj = ctx.enter_context(tc.tile_pool(name="pp_pj", bufs=2, space="PSUM"))
        pp_tr = ctx.enter_context(tc.tile_pool(name="pp_tr", bufs=1, space="PSUM"))
        pp_sc = ctx.enter_context(tc.tile_pool(name="pp_sc", bufs=2, space="PSUM"))
        pp_av = ctx.enter_context(tc.tile_pool(name="pp_av", bufs=2, space="PSUM"))
        pp_bc = ctx.enter_context(tc.tile_pool(name="pp_bc", bufs=1, space="PSUM"))

        # ---- constants ----
        ident = singles.tile([128, 128], F16, name="ident", tag="ident")
        make_identity(nc, ident)

        # mask0[r, c] = 0 where c >= r else NEG (applied to diagonal tiles;
        # a diagonal tile at relative offset k uses mask0[:, : SB - 128 k]
        # against psum cols [128 k :]).
        mask0 = singles.tile([128, SB], F32, name="mask0", tag="mask0")
        nc.gpsimd.memset(mask0, 0.0)
        nc.gpsimd.affine_select(
            out=mask0,
            in_=mask0,
            compare_op=mybir.AluOpType.is_ge,
            fill=NEG,
            base=0,
            pattern=[[1, SB]],
            channel_multiplier=-1,
        )

        ones1 = singles.tile([1, DK], F16, name="ones1", tag="ones1")
        nc.vector.memset(ones1, 1.0)

        sbq = singles.tile([128, NQT], F32, name="sbq", tag="sbq")
        nc.sync.dma_start(out=sbq, in_=bq.rearrange("(t p) -> p t", p=128))
        sbk = singles.tile([128, 1], F32, name="sbk", tag="sbk")
        nc.sync.dma_start(out=sbk, in_=bk.rearrange("(t p) -> p t", p=128))
        sbv = singles.tile([128, 1], F32, name="sbv", tag="sbv")
        nc.sync.dma_start(out=sbv, in_=bv.rearrange("(t p) -> p t", p=128))

        # ---- persistent weights ----
        wq_t = []
        for kd in range(NKD):
            t = wpool.tile([128, QC], F16, name=f"wq{kd}", tag=f"wq{kd}")
            nc.sync.dma_start(out=t, in_=wq[kd * 128:(kd + 1) * 128, :])
            wq_t.append(t)
        wk_t = []
        wv_t = []
        for kd in range(NKD):
            t = wpool.tile([128, KC], F16, name=f"wk{kd}", tag=f"wk{kd}")
            nc.sync.dma_start(out=t, in_=wk[kd * 128:(kd + 1) * 128, :])
            wk_t.append(t)
            t2 = wpool.tile([128, KC], F16, name=f"wv{kd}", tag=f"wv{kd}")
            nc.sync.dma_start(out=t2, in_=wv[kd * 128:(kd + 1) * 128, :])
            wv_t.append(t2)
        wo_t = []
        for pr in range(NPR):
            t = wpool.tile([128, D], F16, name=f"wo{pr}", tag=f"wo{pr}")
            nc.sync.dma_start(out=t, in_=wo[pr * 128:(pr + 1) * 128, :])
            wo_t.append(t)

        # ---- persistent K^T and V_aug ----
        kT_all = wpool.tile([128, S], F16, name="kT_all", tag="kT_all")
        # vaug[g][j][:, tt, 0:64] = V rows for t-chunk (j*TPB+tt), group g;
        # col 64 = ones (folds the softmax denominator into the AV matmul).
        vaug = [[None] * NB for _ in range(GPC)]
        for g in range(GPC):
            for j in range(NB):
                t = wpool.tile(
                    [128, TPB, DK + 1], F16,
                    name=f"vaug{g}_{j}", tag=f"vaug{g}_{j}",
                )
                nc.vector.memset(t[:, :, DK:DK + 1], 1.0)
                vaug[g][j] = t

        # ---- main loop over s-blocks ----
        for j in range(NB):
            s0 = j * SB

            # xT tiles for this block: [128 d, SB s] each.
            xt = []
            for kd in range(NKD):
                t = xtp.tile([128, SB], F16, name=f"xt{kd}", tag=f"xt{kd}")
                nc.sync.dma_start(
                    out=t, in_=xT[kd * 128:(kd + 1) * 128, s0:s0 + SB]
                )
                xt.append(t)

            # Q^T projection: qT[qt] = (Wq_chunk^T @ xT_chunk summed) + bq
            qT = []
            for qt in range(NQT):
                ps = pp_pj.tile([128, SB], F32, name="ps_q", tag="pj")
                for kd in range(NKD):
                    nc.tensor.matmul(
                        out=ps,
                        lhsT=wq_t[kd][:, qt * 128:(qt + 1) * 128],
                        rhs=xt[kd],
                        start=(kd == 0),
                        stop=(kd == NKD - 1),
                    )
                t = qtp.tile([128, SB], F16, name=f"qT{qt}", tag=f"qT{qt}")
                nc.scalar.activation(
                    out=t, in_=ps,
                    func=mybir.ActivationFunctionType.Identity,
                    bias=sbq[:, qt:qt + 1],
                )
                qT.append(t)

            # K^T projection -> persistent kT_all columns [s0:s0+SB].
            ps_k = pp_pj.tile([128, SB], F32, name="ps_k", tag="pj")
            for kd in range(NKD):
                nc.tensor.matmul(
                    out=ps_k, lhsT=wk_t[kd], rhs=xt[kd],
                    start=(kd == 0), stop=(kd == NKD - 1),
                )
            nc.scalar.activation(
                out=kT_all[:, s0:s0 + SB], in_=ps_k,
                func=mybir.ActivationFunctionType.Identity,
                bias=sbk,
            )

            # V^T projection (transient), then PE-transpose into vaug.
            ps_v = pp_pj.tile([128, SB], F32, name="ps_v", tag="pj")
            for kd in range(NKD):
                nc.tensor.matmul(
                    out=ps_v, lhsT=wv_t[kd], rhs=xt[kd],
                    start=(kd == 0), stop=(kd == NKD - 1),
                )
            vT = vtp.tile([128, SB], F16, name="vT", tag="vT")
            nc.scalar.activation(
                out=vT, in_=ps_v,
                func=mybir.ActivationFunctionType.Identity,
                bias=sbv,
            )
            for tt in range(TPB):
                for g in range(GPC):
                    ps_t = pp_tr.tile([128, DK], F16, name="ps_t", tag="tr")
                    nc.tensor.transpose(
                        out=ps_t,
                        in_=vT[g * DK:(g + 1) * DK, tt * 128:(tt + 1) * 128],
                        identity=ident[g * DK:(g + 1) * DK, g * DK:(g + 1) * DK],
                    )
                    nc.vector.tensor_copy(
                        out=vaug[g][j][:, tt, 0:DK], in_=ps_t
                    )

            # Attention per local head.
            nti = TPB * (j + 1)  # t-chunks needed for this block
            # Host permutes Wq cols / Wo rows so q-tile m holds head m
            # (group 0) in partitions 0:64 and head 4+m (group 1) in
            # partitions 64:128 -- q rows then share the base partition of
            # the head's K^T rows, as the PE quadrant tiling requires.
            apairs = []
            for hl in range(HL):
                g = hl // REP
                qrow = g * DK
                qtile = qT[hl % REP]
                ps_av = pp_av.tile([DK + 1, SB], F32, name="ps_av", tag="av")
                for ti in range(nti):
                    krel = ti - TPB * j
                    c0 = 128 * krel if krel > 0 else 0
                    ps_p = pp_sc.tile([128, SB], F32, name="ps_p", tag="sc")
                    nc.tensor.matmul(
                        out=ps_p[:, c0:SB],
                        lhsT=kT_all[g * DK:(g + 1) * DK,
                                    ti * 128:(ti + 1) * 128],
                        rhs=qtile[qrow:qrow + DK, c0:SB],
                        start=True, stop=True,
                    )
                    if krel >= 0:
                        # diagonal tile: additive causal mask
                        nc.vector.tensor_add(
                            out=ps_p[:, c0:SB],
                            in0=ps_p[:, c0:SB],
                            in1=mask0[:, 0:SB - c0],
                        )
                    pt = ptp.tile([128, SB], F16, name="pt", tag="pt")
                    nc.scalar.activation(
                        out=pt[:, c0:SB], in_=ps_p[:, c0:SB],
                        func=mybir.ActivationFunctionType.Exp,
                        scale=0.125,
                    )
                    nc.tensor.matmul(
                        out=ps_av[:, c0:SB],
                        lhsT=vaug[g][ti // TPB][:, ti % TPB, :],
                        rhs=pt[:, c0:SB],
                        start=(ti == 0),
                        stop=(ti == nti - 1),
                    )
                # normalize: out_h^T = ps_av[0:DK] * (1 / ps_av[DK])
                r1 = smp.tile([1, SB], F32, name="r1", tag="r1")
                nc.vector.reciprocal(out=r1, in_=ps_av[DK:DK + 1, :])
                r1h = smp.tile([1, SB], F16, name="r1h", tag="r1h")
                nc.gpsimd.tensor_copy(out=r1h, in_=r1)
                # replicate r1 across 64 partitions via a K=1 PE matmul
                bc = pp_bc.tile([DK, SB], F32, name="bc", tag="bc")
                nc.tensor.matmul(
                    out=bc, lhsT=ones1, rhs=r1h, start=True, stop=True
                )
                pr = hl % REP
                half = hl // REP
                if half == 0:
                    apair = atp.tile(
                        [128, SB], F16, name=f"ap{pr}", tag=f"ap{pr}"
                    )
                    apairs.append(apair)
                av_s = smp.tile([DK, SB], F32, name="av_s", tag="av_s")
                nc.vector.tensor_copy(out=av_s, in_=ps_av[0:DK, :])
                nc.vector.tensor_mul(
                    out=apairs[pr][half * DK:(half + 1) * DK, :],
                    in0=av_s,
                    in1=bc,
                )

            # Output projection: outT[:, s0:s0+SB] partial.
            for ot in range(NOT):
                ps_o = pp_pj.tile([128, SB], F32, name="ps_o", tag="pj")
                for pr in range(NPR):
                    nc.tensor.matmul(
                        out=ps_o,
                        lhsT=wo_t[pr][:, ot * 128:(ot + 1) * 128],
                        rhs=apairs[pr],
                        start=(pr == 0),
                        stop=(pr == NPR - 1),
                    )
                osb = otp.tile([128, SB], F32, name="osb", tag="osb")
                nc.vector.tensor_copy(out=osb, in_=ps_o)
                nc.sync.dma_start(
                    out=outT[ot * 128:(ot + 1) * 128, s0:s0 + SB], in_=osb
                )

    nc.compile()
    return nc


def make_in_maps(x, Wq, bq, Wk, bk, Wv, bv, Wo, bo):
    x = np.asarray(x, dtype=np.float32)
    Wq = np.asarray(Wq, dtype=np.float32)
    Wk = np.asarray(Wk, dtype=np.float32)
    Wv = np.asarray(Wv, dtype=np.float32)
    Wo = np.asarray(Wo, dtype=np.float32)
    bq = np.asarray(bq, dtype=np.float32)
    bk = np.asarray(bk, dtype=np.float32)
    bv = np.asarray(bv, dtype=np.float32)
    # Local-head layout permutation: q-tile m = [head m (g0) | head 4+m (g1)]
    perm = [0, REP, 1, REP + 1, 2, REP + 2, 3, REP + 3][:HL]
    in_maps = []
    for c in range(NCORES):
        b = c // (NCORES // B)
        gp = c % (NCORES // B)
        q0 = gp * QC
        k0 = gp * KC
        qcols = np.concatenate(
            [np.arange(q0 + hl * DK, q0 + (hl + 1) * DK) for hl in perm]
        )
        in_maps.append({
            "xT": np.ascontiguousarray(x[b].T.astype(np.float16)),
            "wq": np.ascontiguousarray(Wq[:, qcols].astype(np.float16)),
            "wk": np.ascontiguousarray(Wk[:, k0:k0 + KC].astype(np.float16)),
            "wv": np.ascontiguousarray(Wv[:, k0:k0 + KC].astype(np.float16)),
            "wo": np.ascontiguousarray(Wo[qcols, :].astype(np.float16)),
            "bq": np.ascontiguousarray(bq[qcols]),
            "bk": np.ascontiguousarray(bk[k0:k0 + KC]),
            "bv": np.ascontiguousarray(bv[k0:k0 + KC]),
        })
    return in_maps


def assemble_output(results, bo):
    bo = np.asarray(bo, dtype=np.float32)
    out = np.zeros((B, S, D), dtype=np.float32)
    for c in range(NCORES):
        b = c // (NCORES // B)
        out[b] += results[c]["outT"].T
    out += bo
    return out


_NC_CACHE = None


def kernel(x, Wq, bq, Wk, bk, Wv, bv, Wo, bo):
    global _NC_CACHE
    from concourse.bass_utils import run_bass_kernel_spmd

    if _NC_CACHE is None:
        _NC_CACHE = build_gqa_nc()
    nc = _NC_CACHE
    in_maps = make_in_maps(x, Wq, bq, Wk, bk, Wv, bv, Wo, bo)
    res = run_bass_kernel_spmd(nc, in_maps, list(range(NCORES))).results
    return assemble_output(res, bo)



# revision 3
# speedup vs baseline: 1.6586x; 1.6586x over previous
"""Grouped-Query Attention kernel for Trainium2, 8-core SPMD — v2.

Problem (full shapes): B=2, S=2048, D=2048, H=32 q-heads, KV=8 kv-heads,
DK=64, REP=4.

Sharding: 16 (batch, kv-group) units over 8 cores -> each core owns one
batch b and 2 adjacent kv-groups (8 query heads, 512 q-cols / 128 kv-cols).
Each core computes its heads' attention output and a partial output
projection against its 512-row slice of Wo; the host sums the 4 partials
per batch and adds bo.

v2 schedule (single interleaved instruction stream per engine):
 - scores per head-PAIR (heads m, m+4 share one [128, 2, 512] PSUM tile,
   one Exp activation covers both halves via a strided AP)
 - causal triangle zeroed post-exp by affine_select on the Pool engine
   (off the Act/DVE critical chain); PSUM banks get exactly one
   start=True, everything else accumulates onto lazily-materialized
   zeros (start=False + skip_group_check)
 - AV computed transposed: out[s,dk] = pt_chunk^T @ V_aug (F=65 per
   matmul instead of F=512); softmax denominator rides along as the
   vaug ones column; normalization via per-partition tensor_scalar_mul;
   PE-transpose back to [dk, s] for the output projection
 - projections of block j+1 and out-projection of earlier blocks are
   chopped into 4-matmul "filler" units and woven between attention
   steps so the PE never stalls on Exp latency
 - inputs land via few large multi-chunk DMAs so the first projection
   matmul starts ~2 us in; outT is written f16, 4 ot-tiles per DMA
"""

from contextlib import ExitStack

import numpy as np

import concourse.bass as bass
import concourse.tile as tile
from concourse import bacc
from concourse import mybir
from concourse.masks import make_identity

F32 = mybir.dt.float32
F16 = mybir.dt.float16

# Full-problem constants (hardcoded per contest contract).
B = 2
S = 2048
D = 2048
H = 32
KV = 8
DK = 64
REP = H // KV          # 4
NCORES = 8

GPC = (KV * B) // NCORES      # kv-groups per core = 2
QC = GPC * REP * DK           # local q cols = 512
KC = GPC * DK                 # local k cols = 128
HL = GPC * REP                # local heads = 8
NPAIR = HL // 2               # head pairs = 4 (pair m = heads m, m+4)
SB = 512                      # s-block size
NB = S // SB                  # 4 blocks
NKD = D // 128                # 16 contraction chunks for projections
NQT = QC // 128               # 4 q-col tiles
NPR = QC // 128               # 4 head-pair tiles (rhs chunks for out proj)
NOT = D // 128                # 16 out-col tiles
TPB = SB // 128               # 4 t-chunks per s-block
NTC = S // 128                # 16 t-chunks total


def build_gqa_nc():
    nc = bacc.Bacc("TRN2", target_bir_lowering=False, debug=False)

    xT = nc.dram_tensor("xT", [D, S], F16, kind="ExternalInput").ap()
    wq = nc.dram_tensor("wq", [D, QC], F16, kind="ExternalInput").ap()
    wk = nc.dram_tensor("wk", [D, KC], F16, kind="ExternalInput").ap()
    wv = nc.dram_tensor("wv", [D, KC], F16, kind="ExternalInput").ap()
    wo = nc.dram_tensor("wo", [QC, D], F16, kind="ExternalInput").ap()
    bq = nc.dram_tensor("bq", [QC], F32, kind="ExternalInput").ap()
    bk = nc.dram_tensor("bk", [KC], F32, kind="ExternalInput").ap()
    bv = nc.dram_tensor("bv", [KC], F32, kind="ExternalInput").ap()
    outT = nc.dram_tensor("outT", [D, S], F16, kind="ExternalOutput").ap()

    xTr = xT.rearrange("(kd p) s -> p kd s", p=128)     # [128, NKD, S]
    wqr = wq.rearrange("(kd p) c -> p kd c", p=128)     # [128, NKD, QC]
    wkr = wk.rearrange("(kd p) c -> p kd c", p=128)
    wvr = wv.rearrange("(kd p) c -> p kd c", p=128)
    wor = wo.rearrange("(pr p) c -> p pr c", p=128)     # [128, NPR, D]

    with tile.TileContext(nc) as tc, ExitStack() as ctx:
        singles = ctx.enter_context(tc.tile_pool(name="singles", bufs=1))
        wpool = ctx.enter_context(tc.tile_pool(name="wpool", bufs=1))
        xtp = ctx.enter_context(tc.tile_pool(name="xtp", bufs=2))
        qtp = ctx.enter_context(tc.tile_pool(name="qtp", bufs=2))
        vtp = ctx.enter_context(tc.tile_pool(name="vtp", bufs=2))
        ptp = ctx.enter_context(tc.tile_pool(name="ptp", bufs=3))
        atp = ctx.enter_context(tc.tile_pool(name="atp", bufs=2))
        aptp = ctx.enter_context(tc.tile_pool(name="aptp", bufs=3))
        osbp = ctx.enter_context(tc.tile_pool(name="osbp", bufs=2))
        rcpp = ctx.enter_context(tc.tile_pool(name="rcpp", bufs=2))

        # PSUM: 8 banks total = sc 2x2 + av 1x2 + pj 1x2.  Transposes borrow
        # "av" (attn finalize) and "pj" (vaug, inside gen_proj) tag slots.
        pp_sc = ctx.enter_context(tc.tile_pool(name="pp_sc", bufs=2, space="PSUM"))
        pp_av = ctx.enter_context(tc.tile_pool(name="pp_av", bufs=2, space="PSUM"))
        pp_pj = ctx.enter_context(tc.tile_pool(name="pp_pj", bufs=2, space="PSUM"))

        # ---- weights/x as few large DMAs; xt(0)/wq first, wo last ----
        xtall = {}

        def emit_xt_dma(j):
            s0 = j * SB
            t = xtp.tile([128, NKD, SB], F16, name=f"xt{j}", tag="xt")
            for c in range(4):
                nc.sync.dma_start(
                    out=t[:, c * 4:(c + 1) * 4, :],
                    in_=xTr[:, c * 4:(c + 1) * 4, s0:s0 + SB],
                )
            xtall[j] = t

        # Startup DMAs issued from four different engines (all idle at t=0)
        # so the transfers run in parallel instead of serializing on SP.
        wkall = wpool.tile([128, NKD, KC], F16, name="wkall", tag="wkall")
        nc.gpsimd.dma_start(out=wkall, in_=wkr)
        emit_xt_dma(0)                       # on nc.sync
        sbk = singles.tile([128, 1], F32, name="sbk", tag="sbk")
        nc.gpsimd.dma_start(out=sbk, in_=bk.rearrange("(t p) -> p t", p=128))
        sbv = singles.tile([128, 1], F32, name="sbv", tag="sbv")
        nc.gpsimd.dma_start(out=sbv, in_=bv.rearrange("(t p) -> p t", p=128))
        sbq = singles.tile([128, NQT], F32, name="sbq", tag="sbq")
        nc.gpsimd.dma_start(out=sbq, in_=bq.rearrange("(t p) -> p t", p=128))
        wqall = wpool.tile([128, NKD, QC], F16, name="wqall", tag="wqall")
        for c in range(4):
            nc.gpsimd.dma_start(
                out=wqall[:, c * 4:(c + 1) * 4, :],
                in_=wqr[:, c * 4:(c + 1) * 4, :],
            )
        wvall = wpool.tile([128, NKD, KC], F16, name="wvall", tag="wvall")
        nc.scalar.dma_start(out=wvall, in_=wvr)

        ident = singles.tile([128, 128], F16, name="ident", tag="ident")
        make_identity(nc, ident)

        # wo loaded after everything needed for block 0 (first use ~60us in)
        woall = wpool.tile([128, NPR, D], F16, name="woall", tag="woall")
        for c in range(2):
            nc.sync.dma_start(
                out=woall[:, c * 2:(c + 1) * 2, :],
                in_=wor[:, c * 2:(c + 1) * 2, :],
            )

        # ---- persistent K^T and V_aug ----
        kT_all = wpool.tile([128, S], F16, name="kT_all", tag="kT_all")
        # vaug[g][ti]: [t=128, DK+1] f16; col DK = ones (folds the softmax
        # denominator into the AV matmul's 65th output column).
        vaug = [[None] * NTC for _ in range(GPC)]
        for g in range(GPC):
            for ti in range(NTC):
                t = wpool.tile(
                    [128, DK + 1], F16,
                    name=f"vaug{g}_{ti}", tag=f"vaug{g}_{ti}",
                )
                nc.vector.memset(t[:, DK:DK + 1], 1.0)
                vaug[g][ti] = t

        qT_blk = {}
        apair_blk = {}

        # PSUM-touching ops must run on DVE (GPSIMD/Pool cannot access PSUM;
        # the Act engine is kept exp-only).
        def rr_engine(i):
            return nc.vector

        # ---------- filler generators (each yield ~= 4 matmuls of PE) ----------
        def gen_proj(j):
            """Q/K/V projections for block j, in order [k, v, v-transposes,
            q0..q3] (attention needs kT/vaug first). Yields between 4-matmul
            units; 28 yields total."""
            xt = xtall[j]
            s0 = j * SB

            ps_k = pp_pj.tile([128, SB], F32, name="ps_k", tag="pj")
            for kd in range(NKD):
                nc.tensor.matmul(
                    out=ps_k, lhsT=wkall[:, kd, :], rhs=xt[:, kd, :],
                    start=(kd == 0), stop=(kd == NKD - 1),
                )
                if kd % 4 == 3 and kd != NKD - 1:
                    yield
            nc.vector.tensor_scalar_add(
                out=kT_all[:, s0:s0 + SB], in0=ps_k, scalar1=sbk)
            yield

            ps_v = pp_pj.tile([128, SB], F32, name="ps_v", tag="pj")
            for kd in range(NKD):
                nc.tensor.matmul(
                    out=ps_v, lhsT=wvall[:, kd, :], rhs=xt[:, kd, :],
                    start=(kd == 0), stop=(kd == NKD - 1),
                )
                if kd % 4 == 3 and kd != NKD - 1:
                    yield
            vT = vtp.tile([128, SB], F16, name="vT", tag="vT")
            nc.vector.tensor_scalar_add(out=vT, in0=ps_v, scalar1=sbv)
            yield
            # PE-transpose V^T into vaug[g][ti]; psum borrowed from "pj" tag.
            for tt in range(TPB):
                ti = j * TPB + tt
                ps_t = pp_pj.tile([128, 128], F16, name="ps_vt", tag="pj")
                for g in range(GPC):
                    nc.tensor.transpose(
                        out=ps_t[:, g * DK:(g + 1) * DK],
                        in_=vT[g * DK:(g + 1) * DK, tt * 128:(tt + 1) * 128],
                        identity=ident[g * DK:(g + 1) * DK, g * DK:(g + 1) * DK],
                    )
                    rr_engine(ti + g).tensor_copy(
                        out=vaug[g][ti][:, 0:DK],
                        in_=ps_t[:, g * DK:(g + 1) * DK],
                    )
                yield

            qT = []
            for qt in range(NQT):
                ps = pp_pj.tile([128, SB], F32, name="ps_q", tag="pj")
                for kd in range(NKD):
                    nc.tensor.matmul(
                        out=ps,
                        lhsT=wqall[:, kd, qt * 128:(qt + 1) * 128],
                        rhs=xt[:, kd, :],
                        start=(kd == 0),
                        stop=(kd == NKD - 1),
                    )
                    if kd % 4 == 3 and kd != NKD - 1:
                        yield
                t = qtp.tile([128, SB], F16, name=f"qT{qt}", tag=f"qT{qt}")
                nc.vector.tensor_scalar_add(
                    out=t, in0=ps, scalar1=sbq[:, qt:qt + 1])
                qT.append(t)
                if qt == 0:
                    qT_blk[j] = qT   # published list grows in place
                yield

        def gen_outproj(j, tail=False):
            """Output projection for block j (consumes apair tiles).

            tail=True issues per-ot DMAs (pipelines the final drain)."""
            s0 = j * SB
            aps = apair_blk[j]
            for oq in range(NOT // 4):
                osb = osbp.tile([128, 4, SB], F16, name="osb", tag="osb")
                for oi in range(4):
                    ot = oq * 4 + oi
                    ps_o = pp_pj.tile([128, SB], F32, name="ps_o", tag="pj")
                    for pr in range(NPR):
                        nc.tensor.matmul(
                            out=ps_o,
                            lhsT=woall[:, pr, ot * 128:(ot + 1) * 128],
                            rhs=aps[pr],
                            start=(pr == 0),
                            stop=(pr == NPR - 1),
                        )
                    rr_engine(ot).tensor_copy(out=osb[:, oi, :], in_=ps_o)
                    if tail:
                        nc.sync.dma_start(
                            out=outT[ot * 128:(ot + 1) * 128, s0:s0 + SB],
                            in_=osb[:, oi, :],
                        )
                    yield
                if not tail:
                    nc.sync.dma_start(
                        out=outT[oq * SB:(oq + 1) * SB, s0:s0 + SB].rearrange(
                            "(i p) c -> p i c", p=128),
                        in_=osb,
                    )

        def emit_av(j, prev, av):
            """AV matmuls for pending exp'd pair tile: out[s,dk] += ptT @ vaug.

            One start=True per av bank (ti==0, sc==0) zeroes the whole bank;
            every other matmul accumulates onto lazily-materialized zeros
            (start=False + skip_group_check)."""
            pt, ti = prev
            krel = ti - TPB * j
            for half in range(2):
                g = half
                for sc in range(TPB):
                    if krel >= 0 and sc < krel:
                        continue  # s-chunk entirely below the diagonal
                    first = (ti == 0 and sc == 0)
                    nc.tensor.matmul(
                        out=av[half][:, sc, :],
                        lhsT=pt[:, half, sc * 128:(sc + 1) * 128],
                        rhs=vaug[g][ti],
                        start=first,
                        stop=True,
                        skip_group_check=not first,
                    )

        # ---------- main schedule ----------
        # Block 0 preamble: drive proj(0) through k, v, v-transposes and q0
        # (16 units); the q1..q3 remainder becomes block-0 filler so
        # attention starts as early as possible.
        proj0 = gen_proj(0)
        proj0_units = 0
        for _ in range(16):
            next(proj0)
            proj0_units += 1

        for j in range(NB):
            nti = TPB * (j + 1)
            if j + 1 < NB:
                emit_xt_dma(j + 1)

            # filler plan: b0: rest-of-P0 + P1 | b1: P2+O0 | b2: P3 | b3: O1+O2
            if j == 0:
                gens = [proj0, gen_proj(1)]
            elif j == 1:
                gens = [gen_proj(2), gen_outproj(0)]
            elif j == 2:
                gens = [gen_proj(3)]
            else:
                gens = [gen_outproj(1), gen_outproj(2)]

            gen_idx = 0

            def emit_filler(n):
                nonlocal gen_idx, proj0_units
                emitted = 0
                while emitted < n and gen_idx < len(gens):
                    try:
                        next(gens[gen_idx])
                        emitted += 1
                        if j == 0 and gen_idx == 0:
                            proj0_units += 1
                    except StopIteration:
                        gen_idx += 1
                return emitted

            # units: proj = 28, outproj = 16.  Back-load the last block
            # (Act-bound tail) by weighting later pairs heavier.
            UNITS = {0: 12 + 28, 1: 44, 2: 28, 3: 32}
            units_total = UNITS[j]
            PAIR_W = {3: [1.0, 1.0, 2.0, 4.0]}.get(j, [1.0] * NPAIR)
            wsum = sum(PAIR_W)

            for m in range(NPAIR):
                if j == 0:
                    # qT[m] (and its whole PE group) must be emitted before
                    # pair m's first scores matmul (in-order PE stream).
                    while proj0_units < 16 + 4 * m and gen_idx == 0:
                        emit_filler(1)
                qtile = qT_blk[j][m]
                av = [None, None]   # psum accumulators for heads A, B
                for half in range(2):
                    av[half] = pp_av.tile(
                        [128, TPB, DK + 1], F32, name=f"av{half}", tag="av"
                    )
                apair = aptp.tile([128, SB], F16, name=f"ap{m}", tag=f"ap{m}")

                filler_acc = 0.0
                filler_per_step = units_total * PAIR_W[m] / (wsum * nti)

                prev = None  # (pt, ti) pending AV
                for ti in range(nti):
                    krel = ti - TPB * j
                    c0 = 128 * krel if krel > 0 else 0
                    psc = pp_sc.tile([128, 2, SB], F32, name="psc", tag="sc")
                    # scores for heads A (half 0, group 0), B (half 1, group 1)
                    for half in range(2):
                        g = half
                        qrow = g * DK
                        kTsl = kT_all[g * DK:(g + 1) * DK,
                                      ti * 128:(ti + 1) * 128]
                        if krel >= 0:
                            # diagonal chunk: one start=True per bank; the
                            # square accumulates onto lazily-materialized
                            # zeros; causal triangle handled post-exp.
                            if c0 + 128 < SB:
                                nc.tensor.matmul(
                                    out=psc[:, half, c0 + 128:SB],
                                    lhsT=kTsl,
                                    rhs=qtile[qrow:qrow + DK, c0 + 128:SB],
                                    start=True, stop=True,
                                )
                                nc.tensor.matmul(
                                    out=psc[:, half, c0:c0 + 128],
                                    lhsT=kTsl,
                                    rhs=qtile[qrow:qrow + DK, c0:c0 + 128],
                                    start=False, stop=True,
                                    skip_group_check=True,
                                )
                            else:
                                nc.tensor.matmul(
                                    out=psc[:, half, c0:c0 + 128],
                                    lhsT=kTsl,
                                    rhs=qtile[qrow:qrow + DK, c0:c0 + 128],
                                    start=True, stop=True,
                                )
                        else:
                            nc.tensor.matmul(
                                out=psc[:, half, :],
                                lhsT=kTsl,
                                rhs=qtile[qrow:qrow + DK, :],
                                start=True, stop=True,
                            )
                    # one exp for both halves (strided AP over the pair tile)
                    pt = ptp.tile([128, 2, SB], F16, name="pt", tag="pt")
                    nc.scalar.activation(
                        out=pt[:, :, c0:SB], in_=psc[:, :, c0:SB],
                        func=mybir.ActivationFunctionType.Exp,
                        scale=0.125,
                    )
                    if krel >= 0:
                        # zero the strictly-below-diagonal triangle of the
                        # diagonal square (Pool engine, off the Act/DVE path)
                        for half in range(2):
                            nc.gpsimd.affine_select(
                                out=pt[:, half, c0:c0 + 128],
                                in_=pt[:, half, c0:c0 + 128],
                                compare_op=mybir.AluOpType.is_ge,
                                fill=0.0,
                                base=0,
                                pattern=[[1, 128]],
                                channel_multiplier=-1,
                            )

                    # fillers between scores(ti) and AV(ti-1)
                    filler_acc += filler_per_step
                    nf = int(filler_acc)
                    if nf:
                        filler_acc -= nf
                        emit_filler(nf)

                    if prev is not None:
                        emit_av(j, prev, av)
                    prev = (pt, ti)
                emit_av(j, prev, av)

                # ---- finalize pair m: normalize, transpose, pack apair ----
                rcp = rcpp.tile([128, 2, TPB], F32, name="rcp", tag="rcp")
                for half in range(2):
                    nc.vector.reciprocal(
                        out=rcp[:, half, :], in_=av[half][:, :, DK]
                    )
                attn = atp.tile([128, 2, TPB, DK], F16, name="attn", tag="attn")
                for half in range(2):
                    for sc in range(TPB):
                        rr_engine(half * TPB + sc).tensor_scalar_mul(
                            out=attn[:, half, sc, :],
                            in0=av[half][:, sc, 0:DK],
                            scalar1=rcp[:, half, sc:sc + 1],
                        )
                for sc in range(TPB):
                    # transpose psum borrowed from the "av" tag (safe: only
                    # allocated after the normalizes consumed the av slots)
                    ps_t = pp_av.tile([128, 128], F16, name="ps_a", tag="av")
                    for half in range(2):
                        nc.tensor.transpose(
                            out=ps_t[half * DK:(half + 1) * DK, :],
                            in_=attn[:, half, sc, :],
                            identity=ident,
                        )
                    rr_engine(sc).tensor_copy(
                        out=apair[:, sc * 128:(sc + 1) * 128], in_=ps_t
                    )
                if m == 0:
                    apair_blk[j] = []
                apair_blk[j].append(apair)
            # drain any unfinished fillers for this block
            while emit_filler(4):
                pass

        # tail: out-projection of the last block
        for _ in gen_outproj(NB - 1, tail=True):
            pass

    nc.compile()
    return nc


def make_in_maps(x, Wq, bq, Wk, bk, Wv, bv, Wo, bo):
    x = np.asarray(x, dtype=np.float32)
    Wq = np.asarray(Wq, dtype=np.float32)
    Wk = np.asarray(Wk, dtype=np.float32)
    Wv = np.asarray(Wv, dtype=np.float32)
    Wo = np.asarray(Wo, dtype=np.float32)
    bq = np.asarray(bq, dtype=np.float32)
    bk = np.asarray(bk, dtype=np.float32)
    bv = np.asarray(bv, dtype=np.float32)
    # Local-head layout permutation: q-tile m = [head m (g0) | head 4+m (g1)]
    perm = [0, REP, 1, REP + 1, 2, REP + 2, 3, REP + 3][:HL]
    in_maps = []
    for c in range(NCORES):
        b = c // (NCORES // B)
        gp = c % (NCORES // B)
        q0 = gp * QC
        k0 = gp * KC
        qcols = np.concatenate(
            [np.arange(q0 + hl * DK, q0 + (hl + 1) * DK) for hl in perm]
        )
        in_maps.append({
            "xT": np.ascontiguousarray(x[b].T.astype(np.float16)),
            "wq": np.ascontiguousarray(Wq[:, qcols].astype(np.float16)),
            "wk": np.ascontiguousarray(Wk[:, k0:k0 + KC].astype(np.float16)),
            "wv": np.ascontiguousarray(Wv[:, k0:k0 + KC].astype(np.float16)),
            "wo": np.ascontiguousarray(Wo[qcols, :].astype(np.float16)),
            "bq": np.ascontiguousarray(bq[qcols]),
            "bk": np.ascontiguousarray(bk[k0:k0 + KC]),
            "bv": np.ascontiguousarray(bv[k0:k0 + KC]),
        })
    return in_maps


def assemble_output(results, bo):
    bo = np.asarray(bo, dtype=np.float32)
    out = np.zeros((B, S, D), dtype=np.float32)
    for c in range(NCORES):
        b = c // (NCORES // B)
        out[b] += results[c]["outT"].T.astype(np.float32)
    out += bo
    return out


_NC_CACHE = None


def kernel(x, Wq, bq, Wk, bk, Wv, bv, Wo, bo):
    global _NC_CACHE
    from concourse.bass_utils import run_bass_kernel_spmd

    if _NC_CACHE is None:
        _NC_CACHE = build_gqa_nc()
    nc = _NC_CACHE
    in_maps = make_in_maps(x, Wq, bq, Wk, bk, Wv, bv, Wo, bo)
    res = run_bass_kernel_spmd(nc, in_maps, list(range(NCORES))).results
    return assemble_output(res, bo)


# revision 4
# speedup vs baseline: 1.6935x; 1.0211x over previous
"""Grouped-Query Attention kernel for Trainium2, 8-core SPMD — v2.

Problem (full shapes): B=2, S=2048, D=2048, H=32 q-heads, KV=8 kv-heads,
DK=64, REP=4.

Sharding: 16 (batch, kv-group) units over 8 cores -> each core owns one
batch b and 2 adjacent kv-groups (8 query heads, 512 q-cols / 128 kv-cols).
Each core computes its heads' attention output and a partial output
projection against its 512-row slice of Wo; the host sums the 4 partials
per batch and adds bo.

v2 schedule (single interleaved instruction stream per engine):
 - scores per head-PAIR (heads m, m+4 share one [128, 2, 512] PSUM tile,
   one Exp activation covers both halves via a strided AP)
 - causal triangle zeroed post-exp by affine_select on the Pool engine
   (off the Act/DVE critical chain); PSUM banks get exactly one
   start=True, everything else accumulates onto lazily-materialized
   zeros (start=False + skip_group_check)
 - AV computed transposed: out[s,dk] = pt_chunk^T @ V_aug (F=65 per
   matmul instead of F=512); softmax denominator rides along as the
   vaug ones column; normalization via per-partition tensor_scalar_mul;
   PE-transpose back to [dk, s] for the output projection
 - projections of block j+1 and out-projection of earlier blocks are
   chopped into 4-matmul "filler" units and woven between attention
   steps so the PE never stalls on Exp latency
 - inputs land via few large multi-chunk DMAs so the first projection
   matmul starts ~2 us in; outT is written f16, 4 ot-tiles per DMA
"""

from contextlib import ExitStack

import numpy as np

import concourse.bass as bass
import concourse.tile as tile
from concourse import bacc
from concourse import mybir
from concourse.masks import make_identity

F32 = mybir.dt.float32
F16 = mybir.dt.float16

# Full-problem constants (hardcoded per contest contract).
B = 2
S = 2048
D = 2048
H = 32
KV = 8
DK = 64
REP = H // KV          # 4
NCORES = 8

GPC = (KV * B) // NCORES      # kv-groups per core = 2
QC = GPC * REP * DK           # local q cols = 512
KC = GPC * DK                 # local k cols = 128
HL = GPC * REP                # local heads = 8
NPAIR = HL // 2               # head pairs = 4 (pair m = heads m, m+4)
SB = 512                      # s-block size
NB = S // SB                  # 4 blocks
NKD = D // 128                # 16 contraction chunks for projections
NQT = QC // 128               # 4 q-col tiles
NPR = QC // 128               # 4 head-pair tiles (rhs chunks for out proj)
NOT = D // 128                # 16 out-col tiles
TPB = SB // 128               # 4 t-chunks per s-block
NTC = S // 128                # 16 t-chunks total


def build_gqa_nc():
    nc = bacc.Bacc("TRN2", target_bir_lowering=False, debug=False)

    xT = nc.dram_tensor("xT", [D, S], F16, kind="ExternalInput").ap()
    wq = nc.dram_tensor("wq", [D, QC], F16, kind="ExternalInput").ap()
    wk = nc.dram_tensor("wk", [D, KC], F16, kind="ExternalInput").ap()
    wv = nc.dram_tensor("wv", [D, KC], F16, kind="ExternalInput").ap()
    wo = nc.dram_tensor("wo", [QC, D], F16, kind="ExternalInput").ap()
    bq = nc.dram_tensor("bq", [QC], F32, kind="ExternalInput").ap()
    bk = nc.dram_tensor("bk", [KC], F32, kind="ExternalInput").ap()
    bv = nc.dram_tensor("bv", [KC], F32, kind="ExternalInput").ap()
    outT = nc.dram_tensor("outT", [D, S], F16, kind="ExternalOutput").ap()

    xTr = xT.rearrange("(kd p) s -> p kd s", p=128)     # [128, NKD, S]
    wqr = wq.rearrange("(kd p) c -> p kd c", p=128)     # [128, NKD, QC]
    wkr = wk.rearrange("(kd p) c -> p kd c", p=128)
    wvr = wv.rearrange("(kd p) c -> p kd c", p=128)
    wor = wo.rearrange("(pr p) c -> p pr c", p=128)     # [128, NPR, D]

    with tile.TileContext(nc) as tc, ExitStack() as ctx:
        singles = ctx.enter_context(tc.tile_pool(name="singles", bufs=1))
        wpool = ctx.enter_context(tc.tile_pool(name="wpool", bufs=1))
        xtp = ctx.enter_context(tc.tile_pool(name="xtp", bufs=2))
        qtp = ctx.enter_context(tc.tile_pool(name="qtp", bufs=2))
        vtp = ctx.enter_context(tc.tile_pool(name="vtp", bufs=2))
        ptp = ctx.enter_context(tc.tile_pool(name="ptp", bufs=3))
        atp = ctx.enter_context(tc.tile_pool(name="atp", bufs=2))
        aptp = ctx.enter_context(tc.tile_pool(name="aptp", bufs=3))
        osbp = ctx.enter_context(tc.tile_pool(name="osbp", bufs=2))
        rcpp = ctx.enter_context(tc.tile_pool(name="rcpp", bufs=2))

        # PSUM: 8 banks total = sc 2x2 + av 1x2 + pj 1x2.  Transposes borrow
        # "av" (attn finalize) and "pj" (vaug, inside gen_proj) tag slots.
        pp_sc = ctx.enter_context(tc.tile_pool(name="pp_sc", bufs=2, space="PSUM"))
        pp_av = ctx.enter_context(tc.tile_pool(name="pp_av", bufs=2, space="PSUM"))
        pp_pj = ctx.enter_context(tc.tile_pool(name="pp_pj", bufs=2, space="PSUM"))

        # ---- weights/x as few large DMAs; xt(0)/wq first, wo last ----
        xtall = {}

        def emit_xt_dma(j):
            s0 = j * SB
            t = xtp.tile([128, NKD, SB], F16, name=f"xt{j}", tag="xt")
            for c in range(4):
                nc.sync.dma_start(
                    out=t[:, c * 4:(c + 1) * 4, :],
                    in_=xTr[:, c * 4:(c + 1) * 4, s0:s0 + SB],
                )
            xtall[j] = t

        # Startup DMAs issued from four different engines (all idle at t=0)
        # so the transfers run in parallel instead of serializing on SP.
        # Startup: k-projection inputs split across the SP and Pool queues so
        # the first matmuls start ~2.7us in and are never DMA-starved.
        wkall = wpool.tile([128, NKD, KC], F16, name="wkall", tag="wkall")
        xt0 = xtp.tile([128, NKD, SB], F16, name="xt0", tag="xt")
        # SP queue: wk c0, xt c0, xt c1, wk c1..c3
        nc.sync.dma_start(out=wkall[:, 0:4, :], in_=wkr[:, 0:4, :])
        for c in range(2):
            nc.sync.dma_start(
                out=xt0[:, c * 4:(c + 1) * 4, :],
                in_=xTr[:, c * 4:(c + 1) * 4, 0:SB],
            )
        for c in range(1, 4):
            nc.sync.dma_start(
                out=wkall[:, c * 4:(c + 1) * 4, :],
                in_=wkr[:, c * 4:(c + 1) * 4, :],
            )
        # Pool queue: biases, xt c2, c3, then wq
        sbk = singles.tile([128, 1], F32, name="sbk", tag="sbk")
        nc.gpsimd.dma_start(out=sbk, in_=bk.rearrange("(t p) -> p t", p=128))
        sbv = singles.tile([128, 1], F32, name="sbv", tag="sbv")
        nc.gpsimd.dma_start(out=sbv, in_=bv.rearrange("(t p) -> p t", p=128))
        sbq = singles.tile([128, NQT], F32, name="sbq", tag="sbq")
        nc.gpsimd.dma_start(out=sbq, in_=bq.rearrange("(t p) -> p t", p=128))
        for c in range(2, 4):
            nc.gpsimd.dma_start(
                out=xt0[:, c * 4:(c + 1) * 4, :],
                in_=xTr[:, c * 4:(c + 1) * 4, 0:SB],
            )
        xtall[0] = xt0
        wqall = wpool.tile([128, NKD, QC], F16, name="wqall", tag="wqall")
        for c in range(4):
            nc.gpsimd.dma_start(
                out=wqall[:, c * 4:(c + 1) * 4, :],
                in_=wqr[:, c * 4:(c + 1) * 4, :],
            )
        wvall = wpool.tile([128, NKD, KC], F16, name="wvall", tag="wvall")
        nc.scalar.dma_start(out=wvall, in_=wvr)

        ident = singles.tile([128, 128], F16, name="ident", tag="ident")
        make_identity(nc, ident)

        # wo loaded after everything needed for block 0 (first use ~60us in)
        woall = wpool.tile([128, NPR, D], F16, name="woall", tag="woall")
        for c in range(2):
            nc.sync.dma_start(
                out=woall[:, c * 2:(c + 1) * 2, :],
                in_=wor[:, c * 2:(c + 1) * 2, :],
            )

        # ---- persistent K^T and V_aug ----
        kT_all = wpool.tile([128, S], F16, name="kT_all", tag="kT_all")
        # vaug[g][ti]: [t=128, DK+1] f16; col DK = ones (folds the softmax
        # denominator into the AV matmul's 65th output column).
        vaug = [[None] * NTC for _ in range(GPC)]
        for g in range(GPC):
            for ti in range(NTC):
                t = wpool.tile(
                    [128, DK + 1], F16,
                    name=f"vaug{g}_{ti}", tag=f"vaug{g}_{ti}",
                )
                nc.vector.memset(t[:, DK:DK + 1], 1.0)
                vaug[g][ti] = t

        qT_blk = {}
        apair_blk = {}

        # PSUM-touching ops must run on DVE (GPSIMD/Pool cannot access PSUM;
        # the Act engine is kept exp-only).
        def rr_engine(i):
            return nc.vector

        # ---------- filler generators (each yield ~= 4 matmuls of PE) ----------
        def gen_proj(j):
            """Q/K/V projections for block j, in order [k, v, v-transposes,
            q0..q3] (attention needs kT/vaug first). Yields between 4-matmul
            units; 28 yields total."""
            xt = xtall[j]
            s0 = j * SB

            ps_k = pp_pj.tile([128, SB], F32, name="ps_k", tag="pj")
            for kd in range(NKD):
                nc.tensor.matmul(
                    out=ps_k, lhsT=wkall[:, kd, :], rhs=xt[:, kd, :],
                    start=(kd == 0), stop=(kd == NKD - 1),
                )
                if kd % 4 == 3 and kd != NKD - 1:
                    yield
            nc.scalar.activation(
                out=kT_all[:, s0:s0 + SB], in_=ps_k,
                func=mybir.ActivationFunctionType.Identity, bias=sbk)
            yield

            ps_v = pp_pj.tile([128, SB], F32, name="ps_v", tag="pj")
            for kd in range(NKD):
                nc.tensor.matmul(
                    out=ps_v, lhsT=wvall[:, kd, :], rhs=xt[:, kd, :],
                    start=(kd == 0), stop=(kd == NKD - 1),
                )
                if kd % 4 == 3 and kd != NKD - 1:
                    yield
            vT = vtp.tile([128, SB], F16, name="vT", tag="vT")
            nc.scalar.activation(
                out=vT, in_=ps_v,
                func=mybir.ActivationFunctionType.Identity, bias=sbv)
            yield
            # PE-transpose V^T into vaug[g][ti]; psum borrowed from "pj" tag.
            for tt in range(TPB):
                ti = j * TPB + tt
                ps_t = pp_pj.tile([128, 128], F16, name="ps_vt", tag="pj")
                for g in range(GPC):
                    nc.tensor.transpose(
                        out=ps_t[:, g * DK:(g + 1) * DK],
                        in_=vT[g * DK:(g + 1) * DK, tt * 128:(tt + 1) * 128],
                        identity=ident[g * DK:(g + 1) * DK, g * DK:(g + 1) * DK],
                    )
                    rr_engine(ti + g).tensor_copy(
                        out=vaug[g][ti][:, 0:DK],
                        in_=ps_t[:, g * DK:(g + 1) * DK],
                    )
                yield

            qT = []
            for qt in range(NQT):
                ps = pp_pj.tile([128, SB], F32, name="ps_q", tag="pj")
                for kd in range(NKD):
                    nc.tensor.matmul(
                        out=ps,
                        lhsT=wqall[:, kd, qt * 128:(qt + 1) * 128],
                        rhs=xt[:, kd, :],
                        start=(kd == 0),
                        stop=(kd == NKD - 1),
                    )
                    if kd % 4 == 3 and kd != NKD - 1:
                        yield
                t = qtp.tile([128, SB], F16, name=f"qT{qt}", tag=f"qT{qt}")
                nc.scalar.activation(
                    out=t, in_=ps,
                    func=mybir.ActivationFunctionType.Identity,
                    bias=sbq[:, qt:qt + 1])
                qT.append(t)
                if qt == 0:
                    qT_blk[j] = qT   # published list grows in place
                yield

        def gen_outproj(j, tail=False):
            """Output projection for block j (consumes apair tiles).

            tail=True issues per-ot DMAs (pipelines the final drain)."""
            s0 = j * SB
            aps = apair_blk[j]
            for oq in range(NOT // 4):
                osb = osbp.tile([128, 4, SB], F16, name="osb", tag="osb")
                for oi in range(4):
                    ot = oq * 4 + oi
                    ps_o = pp_pj.tile([128, SB], F32, name="ps_o", tag="pj")
                    for pr in range(NPR):
                        nc.tensor.matmul(
                            out=ps_o,
                            lhsT=woall[:, pr, ot * 128:(ot + 1) * 128],
                            rhs=aps[pr],
                            start=(pr == 0),
                            stop=(pr == NPR - 1),
                        )
                    if tail and ot % 2 == 1:
                        # tail copies alternate DVE/Act (both idle-ish then;
                        # a single engine would bottleneck the drain)
                        nc.scalar.activation(
                            out=osb[:, oi, :], in_=ps_o,
                            func=mybir.ActivationFunctionType.Copy)
                    else:
                        nc.vector.tensor_copy(out=osb[:, oi, :], in_=ps_o)
                    if tail:
                        nc.sync.dma_start(
                            out=outT[ot * 128:(ot + 1) * 128, s0:s0 + SB],
                            in_=osb[:, oi, :],
                        )
                    yield

                if not tail:
                    nc.sync.dma_start(
                        out=outT[oq * SB:(oq + 1) * SB, s0:s0 + SB].rearrange(
                            "(i p) c -> p i c", p=128),
                        in_=osb,
                    )

        def emit_av(j, prev, av):
            """AV matmuls for pending exp'd pair tile: out[s,dk] += ptT @ vaug.

            One start=True per av bank (ti==0, sc==0) zeroes the whole bank;
            every other matmul accumulates onto lazily-materialized zeros
            (start=False + skip_group_check)."""
            pt, ti = prev
            krel = ti - TPB * j
            for half in range(2):
                g = half
                for sc in range(TPB):
                    if krel >= 0 and sc < krel:
                        continue  # s-chunk entirely below the diagonal
                    first = (ti == 0 and sc == 0)
                    nc.tensor.matmul(
                        out=av[half][:, sc, :],
                        lhsT=pt[:, half, sc * 128:(sc + 1) * 128],
                        rhs=vaug[g][ti],
                        start=first,
                        stop=True,
                        skip_group_check=not first,
                    )

        # ---------- main schedule ----------
        # Block 0 preamble: drive proj(0) through k, v, v-transposes and q0
        # (16 units); the q1..q3 remainder becomes block-0 filler so
        # attention starts as early as possible.
        proj0 = gen_proj(0)
        proj0_units = 0
        for _ in range(16):
            next(proj0)
            proj0_units += 1

        for j in range(NB):
            nti = TPB * (j + 1)
            if j + 1 < NB:
                emit_xt_dma(j + 1)

            # filler plan: b0: rest-of-P0 + P1 | b1: P2+O0 | b2: P3 | b3: O1+O2
            if j == 0:
                gens = [proj0, gen_proj(1)]
            elif j == 1:
                gens = [gen_proj(2), gen_outproj(0)]
            elif j == 2:
                gens = [gen_proj(3)]
            else:
                gens = [gen_outproj(1), gen_outproj(2)]

            gen_idx = 0

            def emit_filler(n):
                nonlocal gen_idx, proj0_units
                emitted = 0
                while emitted < n and gen_idx < len(gens):
                    try:
                        next(gens[gen_idx])
                        emitted += 1
                        if j == 0 and gen_idx == 0:
                            proj0_units += 1
                    except StopIteration:
                        gen_idx += 1
                return emitted

            # units: proj = 28, outproj = 16.
            UNITS = {0: 12 + 28, 1: 44, 2: 28, 3: 32}
            units_total = UNITS[j]
            PAIR_W = [1.0] * NPAIR
            wsum = sum(PAIR_W)

            def emit_pending_transposes():
                """Pair-finalize transposes, deferred so they don't block the
                next pair's scores in the in-order PE stream.  PSUM borrowed
                from the "pj" tag (its ring only interleaves with fillers)."""
                nonlocal pending_tr
                if pending_tr is None:
                    return
                attn_t, apair_t = pending_tr
                pending_tr = None
                ps_t = pp_pj.tile([128, SB], F16, name="ps_at", tag="pj")
                for half in range(2):
                    for sc in range(TPB):
                        nc.tensor.transpose(
                            out=ps_t[half * DK:(half + 1) * DK,
                                     sc * 128:(sc + 1) * 128],
                            in_=attn_t[:, half, sc, :],
                            identity=ident,
                        )
                nc.vector.tensor_copy(out=apair_t, in_=ps_t)

            pending_tr = None
            for m in range(NPAIR):
                if j == 0:
                    # qT[m] (and its whole PE group) must be emitted before
                    # pair m's first scores matmul (in-order PE stream).
                    while proj0_units < 16 + 4 * m and gen_idx == 0:
                        emit_filler(1)
                qtile = qT_blk[j][m]
                av = [None, None]   # psum accumulators for heads A, B
                for half in range(2):
                    av[half] = pp_av.tile(
                        [128, TPB, DK + 1], F32, name=f"av{half}", tag="av"
                    )
                apair = aptp.tile([128, SB], F16, name=f"ap{m}", tag=f"ap{m}")

                filler_acc = 0.0
                filler_per_step = units_total * PAIR_W[m] / (wsum * nti)

                prev = None  # (pt, ti) pending AV
                for ti in range(nti):
                    krel = ti - TPB * j
                    c0 = 128 * krel if krel > 0 else 0
                    psc = pp_sc.tile([128, 2, SB], F32, name="psc", tag="sc")
                    # scores for heads A (half 0, group 0), B (half 1, group 1)
                    for half in range(2):
                        g = half
                        qrow = g * DK
                        kTsl = kT_all[g * DK:(g + 1) * DK,
                                      ti * 128:(ti + 1) * 128]
                        if krel >= 0:
                            # diagonal chunk: one start=True per bank; the
                            # square accumulates onto lazily-materialized
                            # zeros; causal triangle handled post-exp.
                            if c0 + 128 < SB:
                                nc.tensor.matmul(
                                    out=psc[:, half, c0 + 128:SB],
                                    lhsT=kTsl,
                                    rhs=qtile[qrow:qrow + DK, c0 + 128:SB],
                                    start=True, stop=True,
                                )
                                nc.tensor.matmul(
                                    out=psc[:, half, c0:c0 + 128],
                                    lhsT=kTsl,
                                    rhs=qtile[qrow:qrow + DK, c0:c0 + 128],
                                    start=False, stop=True,
                                    skip_group_check=True,
                                )
                            else:
                                nc.tensor.matmul(
                                    out=psc[:, half, c0:c0 + 128],
                                    lhsT=kTsl,
                                    rhs=qtile[qrow:qrow + DK, c0:c0 + 128],
                                    start=True, stop=True,
                                )
                        else:
                            nc.tensor.matmul(
                                out=psc[:, half, :],
                                lhsT=kTsl,
                                rhs=qtile[qrow:qrow + DK, :],
                                start=True, stop=True,
                            )
                    # one exp for both halves (strided AP over the pair tile)
                    pt = ptp.tile([128, 2, SB], F16, name="pt", tag="pt")
                    nc.scalar.activation(
                        out=pt[:, :, c0:SB], in_=psc[:, :, c0:SB],
                        func=mybir.ActivationFunctionType.Exp,
                        scale=0.125,
                    )
                    if krel >= 0:
                        # zero the strictly-below-diagonal triangle of the
                        # diagonal square (Pool engine, off the Act/DVE path)
                        for half in range(2):
                            nc.gpsimd.affine_select(
                                out=pt[:, half, c0:c0 + 128],
                                in_=pt[:, half, c0:c0 + 128],
                                compare_op=mybir.AluOpType.is_ge,
                                fill=0.0,
                                base=0,
                                pattern=[[1, 128]],
                                channel_multiplier=-1,
                            )

                    # fillers between scores(ti) and AV(ti-1)
                    filler_acc += filler_per_step
                    nf = int(filler_acc)
                    if nf:
                        filler_acc -= nf
                        emit_filler(nf)
                    if ti == 1:
                        emit_pending_transposes()

                    if prev is not None:
                        emit_av(j, prev, av)
                    prev = (pt, ti)
                emit_av(j, prev, av)

                # ---- finalize pair m: reciprocal + normalize on DVE; the
                # PE transposes are deferred into the next pair's steps ----
                rcp = rcpp.tile([128, 2, TPB], F32, name="rcp", tag="rcp")
                for half in range(2):
                    nc.vector.reciprocal(
                        out=rcp[:, half, :], in_=av[half][:, :, DK]
                    )
                attn = atp.tile([128, 2, TPB, DK], F16, name="attn", tag="attn")
                for half in range(2):
                    nc.vector.tensor_mul(
                        out=attn[:, half, :, :],
                        in0=av[half][:, :, 0:DK],
                        in1=rcp[:, half, :].unsqueeze(-1).broadcast_to(
                            (128, TPB, DK)),
                    )
                pending_tr = (attn, apair)
                if m == 0:
                    apair_blk[j] = []
                apair_blk[j].append(apair)
            # drain any unfinished fillers for this block
            emit_pending_transposes()
            while emit_filler(4):
                pass

        # tail: out-projection of the last block
        for _ in gen_outproj(NB - 1, tail=True):
            pass

    nc.compile()
    return nc


def make_in_maps(x, Wq, bq, Wk, bk, Wv, bv, Wo, bo):
    x = np.asarray(x, dtype=np.float32)
    Wq = np.asarray(Wq, dtype=np.float32)
    Wk = np.asarray(Wk, dtype=np.float32)
    Wv = np.asarray(Wv, dtype=np.float32)
    Wo = np.asarray(Wo, dtype=np.float32)
    bq = np.asarray(bq, dtype=np.float32)
    bk = np.asarray(bk, dtype=np.float32)
    bv = np.asarray(bv, dtype=np.float32)
    # Local-head layout permutation: q-tile m = [head m (g0) | head 4+m (g1)]
    perm = [0, REP, 1, REP + 1, 2, REP + 2, 3, REP + 3][:HL]
    in_maps = []
    for c in range(NCORES):
        b = c // (NCORES // B)
        gp = c % (NCORES // B)
        q0 = gp * QC
        k0 = gp * KC
        qcols = np.concatenate(
            [np.arange(q0 + hl * DK, q0 + (hl + 1) * DK) for hl in perm]
        )
        in_maps.append({
            "xT": np.ascontiguousarray(x[b].T.astype(np.float16)),
            "wq": np.ascontiguousarray(Wq[:, qcols].astype(np.float16)),
            "wk": np.ascontiguousarray(Wk[:, k0:k0 + KC].astype(np.float16)),
            "wv": np.ascontiguousarray(Wv[:, k0:k0 + KC].astype(np.float16)),
            "wo": np.ascontiguousarray(Wo[qcols, :].astype(np.float16)),
            "bq": np.ascontiguousarray(bq[qcols]),
            "bk": np.ascontiguousarray(bk[k0:k0 + KC]),
            "bv": np.ascontiguousarray(bv[k0:k0 + KC]),
        })
    return in_maps


def assemble_output(results, bo):
    bo = np.asarray(bo, dtype=np.float32)
    out = np.zeros((B, S, D), dtype=np.float32)
    for c in range(NCORES):
        b = c // (NCORES // B)
        out[b] += results[c]["outT"].T.astype(np.float32)
    out += bo
    return out


_NC_CACHE = None


def kernel(x, Wq, bq, Wk, bk, Wv, bv, Wo, bo):
    global _NC_CACHE
    from concourse.bass_utils import run_bass_kernel_spmd

    if _NC_CACHE is None:
        _NC_CACHE = build_gqa_nc()
    nc = _NC_CACHE
    in_maps = make_in_maps(x, Wq, bq, Wk, bk, Wv, bv, Wo, bo)
    res = run_bass_kernel_spmd(nc, in_maps, list(range(NCORES))).results
    return assemble_output(res, bo)


# revision 5
# speedup vs baseline: 1.6982x; 1.0027x over previous
"""Grouped-Query Attention kernel for Trainium2, 8-core SPMD — v2.

Problem (full shapes): B=2, S=2048, D=2048, H=32 q-heads, KV=8 kv-heads,
DK=64, REP=4.

Sharding: 16 (batch, kv-group) units over 8 cores -> each core owns one
batch b and 2 adjacent kv-groups (8 query heads, 512 q-cols / 128 kv-cols).
Each core computes its heads' attention output and a partial output
projection against its 512-row slice of Wo; the host sums the 4 partials
per batch and adds bo.

v2 schedule (single interleaved instruction stream per engine):
 - scores per head-PAIR (heads m, m+4 share one [128, 2, 512] PSUM tile,
   one Exp activation covers both halves via a strided AP)
 - causal triangle zeroed post-exp by affine_select on the Pool engine
   (off the Act/DVE critical chain); PSUM banks get exactly one
   start=True, everything else accumulates onto lazily-materialized
   zeros (start=False + skip_group_check)
 - AV computed transposed: out[s,dk] = pt_chunk^T @ V_aug (F=65 per
   matmul instead of F=512); softmax denominator rides along as the
   vaug ones column; normalization via per-partition tensor_scalar_mul;
   PE-transpose back to [dk, s] for the output projection
 - projections of block j+1 and out-projection of earlier blocks are
   chopped into 4-matmul "filler" units and woven between attention
   steps so the PE never stalls on Exp latency
 - inputs land via few large multi-chunk DMAs so the first projection
   matmul starts ~2 us in; outT is written f16, 4 ot-tiles per DMA
"""

from contextlib import ExitStack

import numpy as np

import concourse.bass as bass
import concourse.tile as tile
from concourse import bacc
from concourse import mybir
from concourse.masks import make_identity

F32 = mybir.dt.float32
F16 = mybir.dt.float16

# Full-problem constants (hardcoded per contest contract).
B = 2
S = 2048
D = 2048
H = 32
KV = 8
DK = 64
REP = H // KV          # 4
NCORES = 8

GPC = (KV * B) // NCORES      # kv-groups per core = 2
QC = GPC * REP * DK           # local q cols = 512
KC = GPC * DK                 # local k cols = 128
HL = GPC * REP                # local heads = 8
NPAIR = HL // 2               # head pairs = 4 (pair m = heads m, m+4)
SB = 512                      # s-block size
NB = S // SB                  # 4 blocks
NKD = D // 128                # 16 contraction chunks for projections
NQT = QC // 128               # 4 q-col tiles
NPR = QC // 128               # 4 head-pair tiles (rhs chunks for out proj)
NOT = D // 128                # 16 out-col tiles
TPB = SB // 128               # 4 t-chunks per s-block
NTC = S // 128                # 16 t-chunks total


def build_gqa_nc():
    nc = bacc.Bacc("TRN2", target_bir_lowering=False, debug=False)

    xT = nc.dram_tensor("xT", [D, S], F16, kind="ExternalInput").ap()
    wq = nc.dram_tensor("wq", [D, QC], F16, kind="ExternalInput").ap()
    wk = nc.dram_tensor("wk", [D, KC], F16, kind="ExternalInput").ap()
    wv = nc.dram_tensor("wv", [D, KC], F16, kind="ExternalInput").ap()
    wo = nc.dram_tensor("wo", [QC, D], F16, kind="ExternalInput").ap()
    bq = nc.dram_tensor("bq", [QC], F32, kind="ExternalInput").ap()
    bk = nc.dram_tensor("bk", [KC], F32, kind="ExternalInput").ap()
    bv = nc.dram_tensor("bv", [KC], F32, kind="ExternalInput").ap()
    outT = nc.dram_tensor("outT", [D, S], F16, kind="ExternalOutput").ap()

    xTr = xT.rearrange("(kd p) s -> p kd s", p=128)     # [128, NKD, S]
    wqr = wq.rearrange("(kd p) c -> p kd c", p=128)     # [128, NKD, QC]
    wkr = wk.rearrange("(kd p) c -> p kd c", p=128)
    wvr = wv.rearrange("(kd p) c -> p kd c", p=128)
    wor = wo.rearrange("(pr p) c -> p pr c", p=128)     # [128, NPR, D]

    with tile.TileContext(nc) as tc, ExitStack() as ctx:
        singles = ctx.enter_context(tc.tile_pool(name="singles", bufs=1))
        wpool = ctx.enter_context(tc.tile_pool(name="wpool", bufs=1))
        xtp = ctx.enter_context(tc.tile_pool(name="xtp", bufs=2))
        qtp = ctx.enter_context(tc.tile_pool(name="qtp", bufs=2))
        vtp = ctx.enter_context(tc.tile_pool(name="vtp", bufs=2))
        ptp = ctx.enter_context(tc.tile_pool(name="ptp", bufs=3))
        atp = ctx.enter_context(tc.tile_pool(name="atp", bufs=2))
        aptp = ctx.enter_context(tc.tile_pool(name="aptp", bufs=3))
        osbp = ctx.enter_context(tc.tile_pool(name="osbp", bufs=2))
        rcpp = ctx.enter_context(tc.tile_pool(name="rcpp", bufs=2))

        # PSUM: 8 banks total = sc 2x2 + av 1x2 + pj 1x2.  Transposes borrow
        # "av" (attn finalize) and "pj" (vaug, inside gen_proj) tag slots.
        pp_sc = ctx.enter_context(tc.tile_pool(name="pp_sc", bufs=2, space="PSUM"))
        pp_av = ctx.enter_context(tc.tile_pool(name="pp_av", bufs=2, space="PSUM"))
        pp_pj = ctx.enter_context(tc.tile_pool(name="pp_pj", bufs=2, space="PSUM"))

        # ---- weights/x as few large DMAs; xt(0)/wq first, wo last ----
        xtall = {}

        def emit_xt_dma(j):
            s0 = j * SB
            t = xtp.tile([128, NKD, SB], F16, name=f"xt{j}", tag="xt")
            for c in range(4):
                nc.sync.dma_start(
                    out=t[:, c * 4:(c + 1) * 4, :],
                    in_=xTr[:, c * 4:(c + 1) * 4, s0:s0 + SB],
                )
            xtall[j] = t

        # Startup DMAs issued from four different engines (all idle at t=0)
        # so the transfers run in parallel instead of serializing on SP.
        # Startup: k-projection inputs split across the SP and Pool queues so
        # the first matmuls start ~2.7us in and are never DMA-starved.
        wkall = wpool.tile([128, NKD, KC], F16, name="wkall", tag="wkall")
        xt0 = xtp.tile([128, NKD, SB], F16, name="xt0", tag="xt")
        # SP queue: wk c0, xt c0, xt c1, wk c1..c3
        nc.sync.dma_start(out=wkall[:, 0:4, :], in_=wkr[:, 0:4, :])
        for c in range(2):
            nc.sync.dma_start(
                out=xt0[:, c * 4:(c + 1) * 4, :],
                in_=xTr[:, c * 4:(c + 1) * 4, 0:SB],
            )
        for c in range(1, 4):
            nc.sync.dma_start(
                out=wkall[:, c * 4:(c + 1) * 4, :],
                in_=wkr[:, c * 4:(c + 1) * 4, :],
            )
        # Pool queue: biases, xt c2, c3, then wq
        sbk = singles.tile([128, 1], F32, name="sbk", tag="sbk")
        nc.gpsimd.dma_start(out=sbk, in_=bk.rearrange("(t p) -> p t", p=128))
        sbv = singles.tile([128, 1], F32, name="sbv", tag="sbv")
        nc.gpsimd.dma_start(out=sbv, in_=bv.rearrange("(t p) -> p t", p=128))
        sbq = singles.tile([128, NQT], F32, name="sbq", tag="sbq")
        nc.gpsimd.dma_start(out=sbq, in_=bq.rearrange("(t p) -> p t", p=128))
        for c in range(2, 4):
            nc.gpsimd.dma_start(
                out=xt0[:, c * 4:(c + 1) * 4, :],
                in_=xTr[:, c * 4:(c + 1) * 4, 0:SB],
            )
        xtall[0] = xt0
        wqall = wpool.tile([128, NKD, QC], F16, name="wqall", tag="wqall")
        for c in range(4):
            nc.gpsimd.dma_start(
                out=wqall[:, c * 4:(c + 1) * 4, :],
                in_=wqr[:, c * 4:(c + 1) * 4, :],
            )
        wvall = wpool.tile([128, NKD, KC], F16, name="wvall", tag="wvall")
        nc.scalar.dma_start(out=wvall, in_=wvr)

        ident = singles.tile([128, 128], F16, name="ident", tag="ident")
        make_identity(nc, ident)

        # wo loaded after everything needed for block 0 (first use ~60us in)
        woall = wpool.tile([128, NPR, D], F16, name="woall", tag="woall")
        for c in range(2):
            nc.sync.dma_start(
                out=woall[:, c * 2:(c + 1) * 2, :],
                in_=wor[:, c * 2:(c + 1) * 2, :],
            )

        # ---- persistent K^T and V_aug ----
        kT_all = wpool.tile([128, S], F16, name="kT_all", tag="kT_all")
        # vaug[g][ti]: [t=128, DK+1] f16; col DK = ones (folds the softmax
        # denominator into the AV matmul's 65th output column).
        vaug = [[None] * NTC for _ in range(GPC)]
        for g in range(GPC):
            for ti in range(NTC):
                t = wpool.tile(
                    [128, DK + 1], F16,
                    name=f"vaug{g}_{ti}", tag=f"vaug{g}_{ti}",
                )
                nc.vector.memset(t[:, DK:DK + 1], 1.0)
                vaug[g][ti] = t

        qT_blk = {}
        apair_blk = {}

        # PSUM-touching ops must run on DVE (GPSIMD/Pool cannot access PSUM;
        # the Act engine is kept exp-only).
        def rr_engine(i):
            return nc.vector

        # ---------- filler generators (each yield ~= 4 matmuls of PE) ----------
        def gen_proj(j):
            """Q/K/V projections for block j, in order [k, q0, v,
            v-transposes, q1..q3] (block-0 attention can start right after
            k+q0). Yields between 4-matmul units; 28 yields total:
            k: 1-4, q0: 5-8, v: 9-12, trs: 13-16, q1: 17-20, q2: 21-24,
            q3: 25-28."""
            xt = xtall[j]
            s0 = j * SB

            ps_k = pp_pj.tile([128, SB], F32, name="ps_k", tag="pj")
            for kd in range(NKD):
                nc.tensor.matmul(
                    out=ps_k, lhsT=wkall[:, kd, :], rhs=xt[:, kd, :],
                    start=(kd == 0), stop=(kd == NKD - 1),
                )
                if kd % 4 == 3 and kd != NKD - 1:
                    yield
            nc.scalar.activation(
                out=kT_all[:, s0:s0 + SB], in_=ps_k,
                func=mybir.ActivationFunctionType.Identity, bias=sbk)
            yield

            qT = []
            qT_blk[j] = qT   # published; grows in place

            def q_group(qt):
                ps = pp_pj.tile([128, SB], F32, name="ps_q", tag="pj")
                for kd in range(NKD):
                    nc.tensor.matmul(
                        out=ps,
                        lhsT=wqall[:, kd, qt * 128:(qt + 1) * 128],
                        rhs=xt[:, kd, :],
                        start=(kd == 0),
                        stop=(kd == NKD - 1),
                    )
                    if kd % 4 == 3 and kd != NKD - 1:
                        yield
                t = qtp.tile([128, SB], F16, name=f"qT{qt}", tag=f"qT{qt}")
                nc.scalar.activation(
                    out=t, in_=ps,
                    func=mybir.ActivationFunctionType.Identity,
                    bias=sbq[:, qt:qt + 1])
                qT.append(t)
                yield

            yield from q_group(0)

            ps_v = pp_pj.tile([128, SB], F32, name="ps_v", tag="pj")
            for kd in range(NKD):
                nc.tensor.matmul(
                    out=ps_v, lhsT=wvall[:, kd, :], rhs=xt[:, kd, :],
                    start=(kd == 0), stop=(kd == NKD - 1),
                )
                if kd % 4 == 3 and kd != NKD - 1:
                    yield
            vT = vtp.tile([128, SB], F16, name="vT", tag="vT")
            nc.scalar.activation(
                out=vT, in_=ps_v,
                func=mybir.ActivationFunctionType.Identity, bias=sbv)
            yield
            # PE-transpose V^T into vaug[g][ti]; psum borrowed from "pj" tag.
            for tt in range(TPB):
                ti = j * TPB + tt
                ps_t = pp_pj.tile([128, 128], F16, name="ps_vt", tag="pj")
                for g in range(GPC):
                    nc.tensor.transpose(
                        out=ps_t[:, g * DK:(g + 1) * DK],
                        in_=vT[g * DK:(g + 1) * DK, tt * 128:(tt + 1) * 128],
                        identity=ident[g * DK:(g + 1) * DK, g * DK:(g + 1) * DK],
                    )
                    rr_engine(ti + g).tensor_copy(
                        out=vaug[g][ti][:, 0:DK],
                        in_=ps_t[:, g * DK:(g + 1) * DK],
                    )
                yield

            for qt in range(1, NQT):
                yield from q_group(qt)

        def gen_outproj(j, tail=False):
            """Output projection for block j (consumes apair tiles).

            tail=True issues per-ot DMAs (pipelines the final drain)."""
            s0 = j * SB
            aps = apair_blk[j]
            for oq in range(NOT // 4):
                osb = osbp.tile([128, 4, SB], F16, name="osb", tag="osb")
                for oi in range(4):
                    ot = oq * 4 + oi
                    ps_o = pp_pj.tile([128, SB], F32, name="ps_o", tag="pj")
                    for pr in range(NPR):
                        nc.tensor.matmul(
                            out=ps_o,
                            lhsT=woall[:, pr, ot * 128:(ot + 1) * 128],
                            rhs=aps[pr],
                            start=(pr == 0),
                            stop=(pr == NPR - 1),
                        )
                    if tail and ot % 2 == 1:
                        # tail copies alternate DVE/Act (both idle-ish then;
                        # a single engine would bottleneck the drain)
                        nc.scalar.activation(
                            out=osb[:, oi, :], in_=ps_o,
                            func=mybir.ActivationFunctionType.Copy)
                    else:
                        nc.vector.tensor_copy(out=osb[:, oi, :], in_=ps_o)
                    if tail:
                        nc.sync.dma_start(
                            out=outT[ot * 128:(ot + 1) * 128, s0:s0 + SB],
                            in_=osb[:, oi, :],
                        )
                    yield

                if not tail:
                    nc.sync.dma_start(
                        out=outT[oq * SB:(oq + 1) * SB, s0:s0 + SB].rearrange(
                            "(i p) c -> p i c", p=128),
                        in_=osb,
                    )

        def emit_av(j, prev, av):
            """AV matmuls for pending exp'd pair tile: out[s,dk] += ptT @ vaug.

            One start=True per av bank (ti==0, sc==0) zeroes the whole bank;
            every other matmul accumulates onto lazily-materialized zeros
            (start=False + skip_group_check)."""
            pt, ti = prev
            krel = ti - TPB * j
            for half in range(2):
                g = half
                for sc in range(TPB):
                    if krel >= 0 and sc < krel:
                        continue  # s-chunk entirely below the diagonal
                    first = (ti == 0 and sc == 0)
                    nc.tensor.matmul(
                        out=av[half][:, sc, :],
                        lhsT=pt[:, half, sc * 128:(sc + 1) * 128],
                        rhs=vaug[g][ti],
                        start=first,
                        stop=True,
                        skip_group_check=not first,
                    )

        # ---------- main schedule ----------
        # Block 0 preamble: drive proj(0) through k, q0, v, v-transposes
        # (16 units); q1..q3 are drained as block-0 fillers before their
        # pairs need them.
        proj0 = gen_proj(0)
        proj0_units = 0
        for _ in range(16):
            next(proj0)
            proj0_units += 1

        pending_tr = None   # deferred pair-finalize transposes (cross-block)

        for j in range(NB):
            nti = TPB * (j + 1)
            if j + 1 < NB:
                emit_xt_dma(j + 1)

            # filler plan: b0: rest-of-P0 + P1 | b1: P2+O0 | b2: P3 | b3: O1+O2
            if j == 0:
                gens = [proj0, gen_proj(1)]
            elif j == 1:
                gens = [gen_proj(2), gen_outproj(0)]
            elif j == 2:
                gens = [gen_proj(3)]
            else:
                gens = [gen_outproj(1), gen_outproj(2)]

            gen_idx = 0

            def emit_filler(n):
                nonlocal gen_idx, proj0_units
                emitted = 0
                while emitted < n and gen_idx < len(gens):
                    try:
                        next(gens[gen_idx])
                        emitted += 1
                        if j == 0 and gen_idx == 0:
                            proj0_units += 1
                    except StopIteration:
                        gen_idx += 1
                return emitted

            # units: proj = 28, outproj = 16.
            UNITS = {0: 12 + 28, 1: 44, 2: 28, 3: 32}
            units_total = UNITS[j]
            PAIR_W = [1.0] * NPAIR
            wsum = sum(PAIR_W)

            def emit_pending_transposes():
                """Pair-finalize transposes, deferred so they don't block the
                next pair's scores in the in-order PE stream.  PSUM borrowed
                from the "pj" tag (its ring only interleaves with fillers)."""
                nonlocal pending_tr
                if pending_tr is None:
                    return
                attn_t, apair_t = pending_tr
                pending_tr = None
                ps_t = pp_pj.tile([128, SB], F16, name="ps_at", tag="pj")
                for half in range(2):
                    for sc in range(TPB):
                        nc.tensor.transpose(
                            out=ps_t[half * DK:(half + 1) * DK,
                                     sc * 128:(sc + 1) * 128],
                            in_=attn_t[:, half, sc, :],
                            identity=ident,
                        )
                nc.vector.tensor_copy(out=apair_t, in_=ps_t)

            # proj(0) progress needed before pair m of block 0 (q0/q1/q2/q3
            # group fully emitted -- in-order PE stream requirement)
            P0_REQ = [16, 20, 24, 28]
            for m in range(NPAIR):
                if j == 0:
                    while proj0_units < P0_REQ[m] and gen_idx == 0:
                        emit_filler(1)
                qtile = qT_blk[j][m]
                av = [None, None]   # psum accumulators for heads A, B
                for half in range(2):
                    av[half] = pp_av.tile(
                        [128, TPB, DK + 1], F32, name=f"av{half}", tag="av"
                    )
                apair = aptp.tile([128, SB], F16, name=f"ap{m}", tag=f"ap{m}")

                filler_acc = 0.0
                filler_per_step = units_total * PAIR_W[m] / (wsum * nti)

                prev = None  # (pt, ti) pending AV
                for ti in range(nti):
                    krel = ti - TPB * j
                    c0 = 128 * krel if krel > 0 else 0
                    psc = pp_sc.tile([128, 2, SB], F32, name="psc", tag="sc")
                    # scores for heads A (half 0, group 0), B (half 1, group 1)
                    for half in range(2):
                        g = half
                        qrow = g * DK
                        kTsl = kT_all[g * DK:(g + 1) * DK,
                                      ti * 128:(ti + 1) * 128]
                        if krel >= 0:
                            # diagonal chunk: one start=True per bank; the
                            # square accumulates onto lazily-materialized
                            # zeros; causal triangle handled post-exp.
                            if c0 + 128 < SB:
                                nc.tensor.matmul(
                                    out=psc[:, half, c0 + 128:SB],
                                    lhsT=kTsl,
                                    rhs=qtile[qrow:qrow + DK, c0 + 128:SB],
                                    start=True, stop=True,
                                )
                                nc.tensor.matmul(
                                    out=psc[:, half, c0:c0 + 128],
                                    lhsT=kTsl,
                                    rhs=qtile[qrow:qrow + DK, c0:c0 + 128],
                                    start=False, stop=True,
                                    skip_group_check=True,
                                )
                            else:
                                nc.tensor.matmul(
                                    out=psc[:, half, c0:c0 + 128],
                                    lhsT=kTsl,
                                    rhs=qtile[qrow:qrow + DK, c0:c0 + 128],
                                    start=True, stop=True,
                                )
                        else:
                            nc.tensor.matmul(
                                out=psc[:, half, :],
                                lhsT=kTsl,
                                rhs=qtile[qrow:qrow + DK, :],
                                start=True, stop=True,
                            )
                    # one exp for both halves (strided AP over the pair tile)
                    pt = ptp.tile([128, 2, SB], F16, name="pt", tag="pt")
                    nc.scalar.activation(
                        out=pt[:, :, c0:SB], in_=psc[:, :, c0:SB],
                        func=mybir.ActivationFunctionType.Exp,
                        scale=0.125,
                    )
                    if krel >= 0:
                        # zero the strictly-below-diagonal triangle of the
                        # diagonal square (Pool engine, off the Act/DVE path)
                        for half in range(2):
                            nc.gpsimd.affine_select(
                                out=pt[:, half, c0:c0 + 128],
                                in_=pt[:, half, c0:c0 + 128],
                                compare_op=mybir.AluOpType.is_ge,
                                fill=0.0,
                                base=0,
                                pattern=[[1, 128]],
                                channel_multiplier=-1,
                            )

                    # fillers between scores(ti) and AV(ti-1)
                    filler_acc += filler_per_step
                    nf = int(filler_acc)
                    if nf:
                        filler_acc -= nf
                        emit_filler(nf)
                    if ti == 1:
                        emit_pending_transposes()

                    if prev is not None:
                        if j == 0 and m == 0:
                            # vaug (v + transposes) must be emitted before
                            # the first AV matmul (in-order PE stream).
                            while proj0_units < 16 and gen_idx == 0:
                                emit_filler(1)
                        emit_av(j, prev, av)
                    prev = (pt, ti)
                emit_av(j, prev, av)

                # ---- finalize pair m: reciprocal + normalize on DVE; the
                # PE transposes are deferred into the next pair's steps ----
                rcp = rcpp.tile([128, 2, TPB], F32, name="rcp", tag="rcp")
                for half in range(2):
                    nc.vector.reciprocal(
                        out=rcp[:, half, :], in_=av[half][:, :, DK]
                    )
                attn = atp.tile([128, 2, TPB, DK], F16, name="attn", tag="attn")
                for half in range(2):
                    nc.vector.tensor_mul(
                        out=attn[:, half, :, :],
                        in0=av[half][:, :, 0:DK],
                        in1=rcp[:, half, :].unsqueeze(-1).broadcast_to(
                            (128, TPB, DK)),
                    )
                pending_tr = (attn, apair)
                if m == 0:
                    apair_blk[j] = []
                apair_blk[j].append(apair)
            # drain any unfinished fillers for this block; the last pair's
            # transposes carry over into the next block's first steps
            # (cross-block deferral), except for the final block.
            if j == NB - 1:
                emit_pending_transposes()
            while emit_filler(4):
                pass

        # tail: out-projection of the last block
        for _ in gen_outproj(NB - 1, tail=True):
            pass

    nc.compile()
    return nc


def make_in_maps(x, Wq, bq, Wk, bk, Wv, bv, Wo, bo):
    x = np.asarray(x, dtype=np.float32)
    Wq = np.asarray(Wq, dtype=np.float32)
    Wk = np.asarray(Wk, dtype=np.float32)
    Wv = np.asarray(Wv, dtype=np.float32)
    Wo = np.asarray(Wo, dtype=np.float32)
    bq = np.asarray(bq, dtype=np.float32)
    bk = np.asarray(bk, dtype=np.float32)
    bv = np.asarray(bv, dtype=np.float32)
    # Local-head layout permutation: q-tile m = [head m (g0) | head 4+m (g1)]
    perm = [0, REP, 1, REP + 1, 2, REP + 2, 3, REP + 3][:HL]
    in_maps = []
    for c in range(NCORES):
        b = c // (NCORES // B)
        gp = c % (NCORES // B)
        q0 = gp * QC
        k0 = gp * KC
        qcols = np.concatenate(
            [np.arange(q0 + hl * DK, q0 + (hl + 1) * DK) for hl in perm]
        )
        in_maps.append({
            "xT": np.ascontiguousarray(x[b].T.astype(np.float16)),
            "wq": np.ascontiguousarray(Wq[:, qcols].astype(np.float16)),
            "wk": np.ascontiguousarray(Wk[:, k0:k0 + KC].astype(np.float16)),
            "wv": np.ascontiguousarray(Wv[:, k0:k0 + KC].astype(np.float16)),
            "wo": np.ascontiguousarray(Wo[qcols, :].astype(np.float16)),
            "bq": np.ascontiguousarray(bq[qcols]),
            "bk": np.ascontiguousarray(bk[k0:k0 + KC]),
            "bv": np.ascontiguousarray(bv[k0:k0 + KC]),
        })
    return in_maps


def assemble_output(results, bo):
    bo = np.asarray(bo, dtype=np.float32)
    out = np.zeros((B, S, D), dtype=np.float32)
    for c in range(NCORES):
        b = c // (NCORES // B)
        out[b] += results[c]["outT"].T.astype(np.float32)
    out += bo
    return out


_NC_CACHE = None


def kernel(x, Wq, bq, Wk, bk, Wv, bv, Wo, bo):
    global _NC_CACHE
    from concourse.bass_utils import run_bass_kernel_spmd

    if _NC_CACHE is None:
        _NC_CACHE = build_gqa_nc()
    nc = _NC_CACHE
    in_maps = make_in_maps(x, Wq, bq, Wk, bk, Wv, bv, Wo, bo)
    res = run_bass_kernel_spmd(nc, in_maps, list(range(NCORES))).results
    return assemble_output(res, bo)
